# revision 6
# baseline (speedup 1.0000x reference)
"""AllegroQeq layer on 8 Trainium2 NeuronCores.

Structure:
  - host: shard 1.6M edges across 8 cores (200704 padded each), build a
    transposed feature stream xw [64, E_P] per core (rows 0:48 = x.T).
  - NEFF-A (per core): per-edge 2-layer MLP -> chis_e [E_P]  (DMA-bound).
  - host mid: bincount(senders) -> chis -> charges/pot/w table, fill
    xw rows 48:64 with w[senders].T.
  - NEFF-B (per core): 3-layer MLP on [x|w_s] + smoothing envelope,
    writes x_out rows in place (PE flip matmuls give row-major output).
  - V passes through untouched.
"""
import os
import sys
import types
import contextlib

sys.path.insert(0, "/opt/trn_rl_repo")

import numpy as np

import concourse.bass as bass
import concourse.bacc as bacc
import concourse.tile as tile
from concourse import mybir
from concourse.bass_utils import run_bass_kernel_spmd

AF = mybir.ActivationFunctionType
ALU = mybir.AluOpType
FP32 = mybir.dt.float32

N_CORES = 8
E = 1_600_000
N_ATOMS = 50_000
F_IN = 48
CE = 16
H = 64
S = 100
E_C = E // N_CORES            # 200000 edges per core
GROUPS = 98                   # groups of 2048 edges
E_P = GROUPS * 2048           # 200704 padded per-core edges

# Profiling side-channel for test.py (not used by the harness).
LAST_EXEC_NS = {}
LAST_RES = {}


def _install_axon_profile_shim():
    """Register the NTFF profile hook missing from the container's antenv
    stub, and neuter the S3 artifact upload. Best-effort."""
    try:
        if "antenv.axon_hooks" not in sys.modules:
            mod = types.ModuleType("antenv.axon_hooks")
            _hook = [None]
            mod.set_axon_ntff_profile_hook = lambda h: _hook.__setitem__(0, h)
            mod.get_axon_ntff_profile_hook = lambda: _hook[0]
            sys.modules["antenv.axon_hooks"] = mod
            import antenv
            antenv.axon_hooks = mod
        from antenv.axon_hooks import set_axon_ntff_profile_hook
        from trn_agent_boot.trn_boot import _ntff_profile_via_ctypes
        hook = _ntff_profile_via_ctypes("/opt/axon/libaxon_pjrt.so")
        if hook is None:
            return False
        set_axon_ntff_profile_hook(hook)
        import concourse.bass_utils as bu
        bu.upload_artifacts = lambda tmpdir: str(tmpdir)
        return True
    except Exception:
        return False


def _build_neff_a():
    """Per-edge chis MLP: chis_e = silu(x @ Wc1) @ Wc2  (Wc* pre-normalized)."""
    nc = bacc.Bacc("TRN2", target_bir_lowering=False, debug=False,
                   num_devices=N_CORES)
    xw = nc.dram_tensor("xw", [H, E_P], FP32, kind="ExternalInput")
    wc1 = nc.dram_tensor("wc1", [F_IN, 32], FP32, kind="ExternalInput")
    wc2 = nc.dram_tensor("wc2", [128, 4], FP32, kind="ExternalInput")
    ce_out = nc.dram_tensor("ce", [GROUPS, 4, 512], FP32, kind="ExternalOutput")

    with tile.TileContext(nc) as tc:
        with tc.tile_pool(name="sbuf", bufs=2) as sb, \
             tc.tile_pool(name="wpool", bufs=1) as wp, \
             tc.tile_pool(name="psum", bufs=2, space="PSUM") as pp:
            wc1_t = wp.tile([F_IN, 32], FP32)
            wc2_t = wp.tile([128, 4], FP32)
            zrow = wp.tile([1, 512], FP32)
            nc.sync.dma_start(wc1_t[:], wc1[:])
            nc.sync.dma_start(wc2_t[:], wc2[:])
            nc.vector.memset(zrow[:], 0.0)

            # pre-zero both psum4 slots so the [128,512] silu reads no junk
            pre = [pp.tile([128, 512], FP32, tag="psum4", name=f"psum4_pre{i}")
                   for i in range(2)]
            for t in pre:
                nc.tensor.matmul(t[:], lhsT=zrow[:, 0:128], rhs=zrow[:],
                                 start=True, stop=True)

            for g in range(GROUPS):
                xs = sb.tile([F_IN, 2048], FP32, tag="xs")
                nc.sync.dma_start(xs[:], xw[0:F_IN, g * 2048:(g + 1) * 2048])
                p4 = pp.tile([128, 512], FP32, tag="psum4")
                for m in range(4):
                    nc.tensor.matmul(p4[32 * m:32 * m + 32, :],
                                     lhsT=wc1_t[:],
                                     rhs=xs[:, 512 * m:512 * m + 512],
                                     start=True, stop=True,
                                     tile_position=(0, 32 * m))
                h1s = sb.tile([128, 512], FP32, tag="h1s")
                nc.scalar.activation(h1s[:], p4[:], AF.Silu)
                pc = pp.tile([4, 512], FP32, tag="pchis")
                nc.tensor.matmul(pc[:], lhsT=wc2_t[:], rhs=h1s[:],
                                 start=True, stop=True)
                stg = sb.tile([4, 512], FP32, tag="cstg")
                nc.vector.tensor_copy(stg[:], pc[:])
                nc.sync.dma_start(ce_out[g][:, :], stg[:])
    nc.compile()
    return nc


def _build_neff_b():
    """3-layer MLP on [x|w_s] with envelope; row-major output via flips."""
    nc = bacc.Bacc("TRN2", target_bir_lowering=False, debug=False,
                   num_devices=N_CORES)
    xw = nc.dram_tensor("xw", [H, E_P], FP32, kind="ExternalInput")
    vecw = nc.dram_tensor("vecw", [128, GROUPS * 48], FP32, kind="ExternalInput")
    w1 = nc.dram_tensor("w1", [H, H], FP32, kind="ExternalInput")
    w2 = nc.dram_tensor("w2", [H, H], FP32, kind="ExternalInput")
    w3 = nc.dram_tensor("w3", [H, H], FP32, kind="ExternalInput")
    xout = nc.dram_tensor("xout", [4 * GROUPS, 128, 256], FP32,
                          kind="ExternalOutput")

    with tile.TileContext(nc) as tc:
        with tc.tile_pool(name="sbuf", bufs=2) as sb, \
             tc.tile_pool(name="wpool", bufs=1) as wp, \
             tc.tile_pool(name="psum", bufs=2, space="PSUM") as pp:
            w1_t = wp.tile([H, H], FP32)
            w2_t = wp.tile([H, H], FP32)
            w3_t = wp.tile([H, H], FP32)
            nc.sync.dma_start(w1_t[:], w1[:])
            nc.sync.dma_start(w2_t[:], w2[:])
            nc.sync.dma_start(w3_t[:], w3[:])

            for g in range(GROUPS):
                xs = sb.tile([H, 2048], FP32, tag="xs")
                nc.sync.dma_start(xs[:], xw[:, g * 2048:(g + 1) * 2048])
                vt = sb.tile([128, 48], FP32, tag="vt")
                nc.sync.dma_start(vt[:], vecw[:, g * 48:(g + 1) * 48])

                # envelope: u = ||r||; env = 1 + u^6 (48u - 21u^2 - 28)
                sq = sb.tile([128, 48], FP32, tag="sq")
                nc.vector.tensor_tensor(sq[:], vt[:], vt[:], op=ALU.mult)
                u = sb.tile([128, 16], FP32, tag="u")
                nc.vector.tensor_reduce(
                    u[:], sq[:].rearrange("p (e c) -> p e c", c=3),
                    axis=mybir.AxisListType.X, op=ALU.add)
                nc.scalar.activation(u[:], u[:], AF.Sqrt)
                nc.vector.tensor_scalar_min(u[:], u[:], 1.0)
                u2 = sb.tile([128, 16], FP32, tag="u2")
                nc.vector.tensor_tensor(u2[:], u[:], u[:], op=ALU.mult)
                u6 = sb.tile([128, 16], FP32, tag="u6")
                nc.vector.tensor_tensor(u6[:], u2[:], u2[:], op=ALU.mult)
                nc.vector.tensor_tensor(u6[:], u6[:], u2[:], op=ALU.mult)
                inner = sb.tile([128, 16], FP32, tag="inner")
                nc.vector.tensor_scalar(inner[:], u[:], -21.0, 48.0,
                                        op0=ALU.mult, op1=ALU.add)
                nc.vector.tensor_tensor(inner[:], inner[:], u[:], op=ALU.mult)
                nc.vector.tensor_scalar_add(inner[:], inner[:], -28.0)
                env = sb.tile([128, 16], FP32, tag="env")
                nc.vector.tensor_tensor(env[:], u6[:], inner[:], op=ALU.mult)
                nc.vector.tensor_scalar(env[:], env[:], 1.0, None, op0=ALU.add)

                for c in range(4):
                    p0 = pp.tile([H, 512], FP32, tag="p0")
                    nc.tensor.matmul(p0[:], lhsT=w1_t[:],
                                     rhs=xs[:, 512 * c:512 * c + 512],
                                     start=True, stop=True)
                    h1 = sb.tile([H, 512], FP32, tag="h1")
                    nc.scalar.activation(h1[:], p0[:], AF.Silu)
                    p1 = pp.tile([H, 512], FP32, tag="p1")
                    nc.tensor.matmul(p1[:], lhsT=w2_t[:], rhs=h1[:],
                                     start=True, stop=True)
                    h2 = sb.tile([H, 512], FP32, tag="h2")
                    nc.scalar.activation(h2[:], p1[:], AF.Silu)
                    h2r = h2[:].rearrange("f (m t) -> f t m", t=4)
                    p2 = pp.tile([128, 256], FP32, tag="p2")
                    stg = sb.tile([128, 256], FP32, tag="stg")
                    for t in range(4):
                        nc.tensor.matmul(p2[:, 64 * t:64 * t + 64],
                                         lhsT=h2r[:, t, :], rhs=w3_t[:],
                                         start=True, stop=True)
                        nc.vector.tensor_scalar_mul(
                            stg[:, 64 * t:64 * t + 64],
                            p2[:, 64 * t:64 * t + 64],
                            env[:, 4 * c + t:4 * c + t + 1])
                    nc.sync.dma_start(xout[4 * g + c][:, :], stg[:])
    nc.compile()
    return nc


def _softplus(x):
    x = x.astype(np.float64)
    return np.log1p(np.exp(-np.abs(x))) + np.maximum(x, 0.0)


def kernel(vectors, x, V, senders, species, radius, hardness, charge_embed,
           chi_scale, g_scale, g_shift, W_chi1, W_chi2, W_w, W_x1, W_x2, W_x3):
    vectors = np.asarray(vectors, np.float32)
    x = np.asarray(x, np.float32)
    V = np.asarray(V)
    senders = np.asarray(senders)
    species = np.asarray(species)

    trace = bool(os.environ.get("BASS_KERNEL_TRACE"))
    if trace:
        trace = _install_axon_profile_shim()

    # ---- pre-normalized weights (match reference's W / sqrt(fan_in)) ----
    wc1n = (np.asarray(W_chi1, np.float32)
            / np.float32(np.sqrt(F_IN))).astype(np.float32)       # [48,16]
    wc2n = (np.asarray(W_chi2, np.float32) / np.float32(np.sqrt(CE))
            * np.float32(chi_scale)).astype(np.float32)           # [16,1]
    w1n = (np.asarray(W_x1, np.float32) / np.float32(np.sqrt(H))).astype(np.float32)
    w2n = (np.asarray(W_x2, np.float32) / np.float32(np.sqrt(H))).astype(np.float32)
    w3n = (np.asarray(W_x3, np.float32) / np.float32(np.sqrt(H))).astype(np.float32)
    wwn = (np.asarray(W_w, np.float32) / np.float32(np.sqrt(1 + CE))).astype(np.float32)

    wc1_pad = np.zeros((F_IN, 32), np.float32)
    wc1_pad[:, :CE] = wc1n
    wc2_blk = np.zeros((128, 4), np.float32)
    for m in range(4):
        wc2_blk[32 * m:32 * m + CE, m] = wc2n[:, 0]

    # ---- shard + host layouts ----
    xw_list, vecw_list, send_pad = [], [], []
    for c in range(N_CORES):
        sl = slice(c * E_C, (c + 1) * E_C)
        xw = np.zeros((H, E_P), np.float32)
        xw[0:F_IN, :E_C] = x[sl].T
        xw_list.append(xw)
        vp = np.zeros((E_P, 3), np.float32)
        vp[:E_C] = vectors[sl]
        # vecw[p, g*48 + 4*cc + tt ... ] <- layout [128, G, 4cc, 4tt, 3]
        v5 = vp.reshape(GROUPS, 4, 128, 4, 3).transpose(2, 0, 1, 3, 4)
        vecw_list.append(np.ascontiguousarray(v5).reshape(128, GROUPS * 48))
        sp = np.zeros(E_P, np.int64)
        sp[:E_C] = senders[sl]
        send_pad.append(sp)

    # ---- NEFF A ----
    ncA = _build_neff_a()
    in_a = [{"xw": xw_list[c], "wc1": wc1_pad, "wc2": wc2_blk}
            for c in range(N_CORES)]
    resA = run_bass_kernel_spmd(ncA, in_a, core_ids=list(range(N_CORES)),
                                trace=trace)
    if trace:
        LAST_EXEC_NS["A"] = resA.exec_time_ns
        LAST_RES["A"] = resA

    # ---- host mid: segment sum + charge solve + w table ----
    chis = np.zeros(N_ATOMS, np.float64)
    for c in range(N_CORES):
        ce = resA.results[c]["ce"].reshape(E_P)
        chis += np.bincount(send_pad[c], weights=ce.astype(np.float64),
                            minlength=N_ATOMS)
    chis = chis.astype(np.float32)

    gammas = (np.asarray(radius, np.float32)[species] * np.float32(g_scale)
              + np.float32(g_shift))
    hard = _softplus(np.asarray(hardness, np.float32)[species]).astype(np.float32)
    eta = (hard.astype(np.float64) + 1.0 / gammas.astype(np.float64))
    q = (-chis.astype(np.float64) / eta)
    pot = np.float32((chis.astype(np.float64) * q + 0.5 * eta * q * q).sum())
    charges = q.astype(np.float32)

    ce_at = np.asarray(charge_embed, np.float32)[species]          # [N,16]
    w_in = np.concatenate([charges[:, None], ce_at], axis=1)       # [N,17]
    w_tab = (w_in @ wwn).astype(np.float32)                        # [N,16]

    for c in range(N_CORES):
        xw_list[c][F_IN:H, :] = w_tab[send_pad[c]].T
        xw_list[c][F_IN:H, E_C:] = 0.0

    # ---- NEFF B ----
    ncB = _build_neff_b()
    in_b = [{"xw": xw_list[c], "vecw": vecw_list[c],
             "w1": w1n, "w2": w2n, "w3": w3n} for c in range(N_CORES)]
    resB = run_bass_kernel_spmd(ncB, in_b, core_ids=list(range(N_CORES)),
                                trace=trace)
    if trace:
        LAST_EXEC_NS["B"] = resB.exec_time_ns
        LAST_RES["B"] = resB

    x_out = np.empty((E, H), np.float32)
    for c in range(N_CORES):
        xo = resB.results[c]["xout"].reshape(E_P, H)
        x_out[c * E_C:(c + 1) * E_C] = xo[:E_C]

    return x_out, V, charges, pot


# revision 7
# speedup vs baseline: 2.9295x; 2.9295x over previous
"""AllegroQeq layer on 8 Trainium2 NeuronCores.

Structure:
  - host: shard 1.6M edges across 8 cores (200704 padded each), build a
    transposed feature stream xw [64, E_P] per core (rows 0:48 = x.T).
  - NEFF-A (per core): per-edge 2-layer MLP -> chis_e [E_P]  (DMA-bound).
  - host mid: bincount(senders) -> chis -> charges/pot/w table, fill
    xw rows 48:64 with w[senders].T.
  - NEFF-B (per core): 3-layer MLP on [x|w_s] + smoothing envelope,
    writes x_out rows in place (PE flip matmuls give row-major output).
  - V passes through untouched.
"""
import os
import sys
import types
import contextlib

sys.path.insert(0, "/opt/trn_rl_repo")

import numpy as np

import concourse.bass as bass
import concourse.bacc as bacc
import concourse.tile as tile
from concourse import mybir
from concourse.bass_utils import run_bass_kernel_spmd

import ml_dtypes

AF = mybir.ActivationFunctionType
ALU = mybir.AluOpType
FP32 = mybir.dt.float32
BF16 = mybir.dt.bfloat16
NPBF16 = ml_dtypes.bfloat16

N_CORES = 8
E = 1_600_000
N_ATOMS = 50_000
F_IN = 48
CE = 16
H = 64
S = 100
E_C = E // N_CORES            # 200000 edges per core
GROUPS = 98                   # groups of 2048 edges
E_P = GROUPS * 2048           # 200704 padded per-core edges

# Profiling side-channel for test.py (not used by the harness).
LAST_EXEC_NS = {}
LAST_RES = {}


def _install_axon_profile_shim():
    """Register the NTFF profile hook missing from the container's antenv
    stub, and neuter the S3 artifact upload. Best-effort."""
    try:
        if "antenv.axon_hooks" not in sys.modules:
            mod = types.ModuleType("antenv.axon_hooks")
            _hook = [None]
            mod.set_axon_ntff_profile_hook = lambda h: _hook.__setitem__(0, h)
            mod.get_axon_ntff_profile_hook = lambda: _hook[0]
            sys.modules["antenv.axon_hooks"] = mod
            import antenv
            antenv.axon_hooks = mod
        from antenv.axon_hooks import set_axon_ntff_profile_hook
        from trn_agent_boot.trn_boot import _ntff_profile_via_ctypes
        hook = _ntff_profile_via_ctypes("/opt/axon/libaxon_pjrt.so")
        if hook is None:
            return False
        set_axon_ntff_profile_hook(hook)
        import concourse.bass_utils as bu
        bu.upload_artifacts = lambda tmpdir: str(tmpdir)
        return True
    except Exception:
        return False


def _build_neff_a():
    """Per-edge chis MLP: chis_e = silu(x @ Wc1) @ Wc2  (Wc* pre-normalized)."""
    nc = bacc.Bacc("TRN2", target_bir_lowering=False, debug=False,
                   num_devices=N_CORES)
    xw = nc.dram_tensor("xw", [H, E_P], BF16, kind="ExternalInput")
    wc1 = nc.dram_tensor("wc1", [F_IN, 32], BF16, kind="ExternalInput")
    wc2 = nc.dram_tensor("wc2", [128, 4], BF16, kind="ExternalInput")
    ce_out = nc.dram_tensor("ce", [GROUPS, 4, 512], FP32, kind="ExternalOutput")

    with tile.TileContext(nc) as tc:
        with tc.tile_pool(name="sbuf", bufs=3) as sb, \
             tc.tile_pool(name="wpool", bufs=1) as wp, \
             tc.tile_pool(name="psum", bufs=2, space="PSUM") as pp:
            wc1_t = wp.tile([F_IN, 32], BF16)
            wc2_t = wp.tile([128, 4], BF16)
            zrow = wp.tile([1, 512], BF16)
            nc.sync.dma_start(wc1_t[:], wc1[:])
            nc.sync.dma_start(wc2_t[:], wc2[:])
            nc.vector.memset(zrow[:], 0.0)

            # pre-zero both psum4 slots so the [128,512] silu reads no junk
            pre = [pp.tile([128, 512], FP32, tag="psum4", name=f"psum4_pre{i}")
                   for i in range(2)]
            for t in pre:
                nc.tensor.matmul(t[:], lhsT=zrow[:, 0:128], rhs=zrow[:],
                                 start=True, stop=True)

            for g in range(GROUPS):
                xs = sb.tile([F_IN, 2048], BF16, tag="xs")
                nc.sync.dma_start(xs[:], xw[0:F_IN, g * 2048:(g + 1) * 2048])
                p4 = pp.tile([128, 512], FP32, tag="psum4")
                for m in range(4):
                    nc.tensor.matmul(p4[32 * m:32 * m + 32, :],
                                     lhsT=wc1_t[:],
                                     rhs=xs[:, 512 * m:512 * m + 512],
                                     start=True, stop=True,
                                     tile_position=(0, 32 * m))
                h1s = sb.tile([128, 512], BF16, tag="h1s")
                nc.scalar.activation(h1s[:], p4[:], AF.Silu)
                pc = pp.tile([4, 512], FP32, tag="pchis")
                nc.tensor.matmul(pc[:], lhsT=wc2_t[:], rhs=h1s[:],
                                 start=True, stop=True)
                stg = sb.tile([4, 512], FP32, tag="cstg")
                nc.vector.tensor_copy(stg[:], pc[:])
                nc.sync.dma_start(ce_out[g][:, :], stg[:])
    nc.compile()
    return nc


def _build_neff_b():
    """3-layer MLP on [x|w_s] with envelope; row-major output via flips."""
    nc = bacc.Bacc("TRN2", target_bir_lowering=False, debug=False,
                   num_devices=N_CORES)
    xw = nc.dram_tensor("xw", [H, E_P], BF16, kind="ExternalInput")
    vecw = nc.dram_tensor("vecw", [128, GROUPS * 48], FP32, kind="ExternalInput")
    w1 = nc.dram_tensor("w1", [H, H], BF16, kind="ExternalInput")
    w2 = nc.dram_tensor("w2", [H, H], BF16, kind="ExternalInput")
    w3 = nc.dram_tensor("w3", [H, H], BF16, kind="ExternalInput")
    xout = nc.dram_tensor("xout", [4 * GROUPS, 128, 256], FP32,
                          kind="ExternalOutput")

    with tile.TileContext(nc) as tc:
        with tc.tile_pool(name="sbuf", bufs=3) as sb, \
             tc.tile_pool(name="wpool", bufs=1) as wp, \
             tc.tile_pool(name="psum", bufs=2, space="PSUM") as pp:
            w1_t = wp.tile([H, H], BF16)
            w2_t = wp.tile([H, H], BF16)
            w3_t = wp.tile([H, H], BF16)
            nc.sync.dma_start(w1_t[:], w1[:])
            nc.sync.dma_start(w2_t[:], w2[:])
            nc.sync.dma_start(w3_t[:], w3[:])

            for g in range(GROUPS):
                xs = sb.tile([H, 2048], BF16, tag="xs")
                nc.sync.dma_start(xs[:], xw[:, g * 2048:(g + 1) * 2048])
                vt = sb.tile([128, 48], FP32, tag="vt")
                nc.sync.dma_start(vt[:], vecw[:, g * 48:(g + 1) * 48])

                # envelope: u = ||r||; env = 1 + u^6 (48u - 21u^2 - 28)
                sq = sb.tile([128, 48], FP32, tag="sq")
                nc.vector.tensor_tensor(sq[:], vt[:], vt[:], op=ALU.mult)
                u = sb.tile([128, 16], FP32, tag="u")
                nc.vector.tensor_reduce(
                    u[:], sq[:].rearrange("p (e c) -> p e c", c=3),
                    axis=mybir.AxisListType.X, op=ALU.add)
                nc.scalar.activation(u[:], u[:], AF.Sqrt)
                nc.vector.tensor_scalar_min(u[:], u[:], 1.0)
                u2 = sb.tile([128, 16], FP32, tag="u2")
                nc.vector.tensor_tensor(u2[:], u[:], u[:], op=ALU.mult)
                u6 = sb.tile([128, 16], FP32, tag="u6")
                nc.vector.tensor_tensor(u6[:], u2[:], u2[:], op=ALU.mult)
                nc.vector.tensor_tensor(u6[:], u6[:], u2[:], op=ALU.mult)
                inner = sb.tile([128, 16], FP32, tag="inner")
                nc.vector.tensor_scalar(inner[:], u[:], -21.0, 48.0,
                                        op0=ALU.mult, op1=ALU.add)
                nc.vector.tensor_tensor(inner[:], inner[:], u[:], op=ALU.mult)
                nc.vector.tensor_scalar_add(inner[:], inner[:], -28.0)
                env = sb.tile([128, 16], FP32, tag="env")
                nc.vector.tensor_tensor(env[:], u6[:], inner[:], op=ALU.mult)
                nc.vector.tensor_scalar(env[:], env[:], 1.0, None, op0=ALU.add)

                for c in range(4):
                    p0 = pp.tile([H, 512], FP32, tag="p0")
                    nc.tensor.matmul(p0[:], lhsT=w1_t[:],
                                     rhs=xs[:, 512 * c:512 * c + 512],
                                     start=True, stop=True)
                    h1 = sb.tile([H, 512], BF16, tag="h1")
                    nc.scalar.activation(h1[:], p0[:], AF.Silu)
                    p1 = pp.tile([H, 512], FP32, tag="p1")
                    nc.tensor.matmul(p1[:], lhsT=w2_t[:], rhs=h1[:],
                                     start=True, stop=True)
                    h2 = sb.tile([H, 512], BF16, tag="h2")
                    nc.scalar.activation(h2[:], p1[:], AF.Silu)
                    h2r = h2[:].rearrange("f (m t) -> f t m", t=4)
                    p2 = pp.tile([128, 256], FP32, tag="p2")
                    stg = sb.tile([128, 256], FP32, tag="stg")
                    for t in range(4):
                        nc.tensor.matmul(p2[:, 64 * t:64 * t + 64],
                                         lhsT=h2r[:, t, :], rhs=w3_t[:],
                                         start=True, stop=True)
                        nc.vector.tensor_scalar_mul(
                            stg[:, 64 * t:64 * t + 64],
                            p2[:, 64 * t:64 * t + 64],
                            env[:, 4 * c + t:4 * c + t + 1])
                    nc.sync.dma_start(xout[4 * g + c][:, :], stg[:])
    nc.compile()
    return nc


def _softplus(x):
    x = x.astype(np.float64)
    return np.log1p(np.exp(-np.abs(x))) + np.maximum(x, 0.0)


def kernel(vectors, x, V, senders, species, radius, hardness, charge_embed,
           chi_scale, g_scale, g_shift, W_chi1, W_chi2, W_w, W_x1, W_x2, W_x3):
    vectors = np.asarray(vectors, np.float32)
    x = np.asarray(x, np.float32)
    V = np.asarray(V)
    senders = np.asarray(senders)
    species = np.asarray(species)

    trace = bool(os.environ.get("BASS_KERNEL_TRACE"))
    if trace:
        trace = _install_axon_profile_shim()

    # ---- pre-normalized weights (match reference's W / sqrt(fan_in)) ----
    wc1n = (np.asarray(W_chi1, np.float32)
            / np.float32(np.sqrt(F_IN))).astype(np.float32)       # [48,16]
    wc2n = (np.asarray(W_chi2, np.float32) / np.float32(np.sqrt(CE))
            * np.float32(chi_scale)).astype(np.float32)           # [16,1]
    w1n = (np.asarray(W_x1, np.float32) / np.float32(np.sqrt(H))).astype(np.float32)
    w2n = (np.asarray(W_x2, np.float32) / np.float32(np.sqrt(H))).astype(np.float32)
    w3n = (np.asarray(W_x3, np.float32) / np.float32(np.sqrt(H))).astype(np.float32)
    wwn = (np.asarray(W_w, np.float32) / np.float32(np.sqrt(1 + CE))).astype(np.float32)

    wc1_pad = np.zeros((F_IN, 32), NPBF16)
    wc1_pad[:, :CE] = wc1n.astype(NPBF16)
    wc2_blk = np.zeros((128, 4), NPBF16)
    for m in range(4):
        wc2_blk[32 * m:32 * m + CE, m] = wc2n[:, 0].astype(NPBF16)
    w1b = w1n.astype(NPBF16)
    w2b = w2n.astype(NPBF16)
    w3b = w3n.astype(NPBF16)

    # ---- shard + host layouts ----
    xw_list, vecw_list, send_pad = [], [], []
    for c in range(N_CORES):
        sl = slice(c * E_C, (c + 1) * E_C)
        xw = np.zeros((H, E_P), NPBF16)
        xw[0:F_IN, :E_C] = x[sl].T.astype(NPBF16)
        xw_list.append(xw)
        vp = np.zeros((E_P, 3), np.float32)
        vp[:E_C] = vectors[sl]
        # vecw[p, g*48 + 4*cc + tt ... ] <- layout [128, G, 4cc, 4tt, 3]
        v5 = vp.reshape(GROUPS, 4, 128, 4, 3).transpose(2, 0, 1, 3, 4)
        vecw_list.append(np.ascontiguousarray(v5).reshape(128, GROUPS * 48))
        sp = np.zeros(E_P, np.int64)
        sp[:E_C] = senders[sl]
        send_pad.append(sp)

    # ---- NEFF A ----
    ncA = _build_neff_a()
    in_a = [{"xw": xw_list[c], "wc1": wc1_pad, "wc2": wc2_blk}
            for c in range(N_CORES)]
    resA = run_bass_kernel_spmd(ncA, in_a, core_ids=list(range(N_CORES)),
                                trace=trace)
    if trace:
        LAST_EXEC_NS["A"] = resA.exec_time_ns
        LAST_RES["A"] = resA

    # ---- host mid: segment sum + charge solve + w table ----
    chis = np.zeros(N_ATOMS, np.float64)
    for c in range(N_CORES):
        ce = resA.results[c]["ce"].reshape(E_P)
        chis += np.bincount(send_pad[c], weights=ce.astype(np.float64),
                            minlength=N_ATOMS)
    chis = chis.astype(np.float32)

    gammas = (np.asarray(radius, np.float32)[species] * np.float32(g_scale)
              + np.float32(g_shift))
    hard = _softplus(np.asarray(hardness, np.float32)[species]).astype(np.float32)
    eta = (hard.astype(np.float64) + 1.0 / gammas.astype(np.float64))
    q = (-chis.astype(np.float64) / eta)
    pot = np.float32((chis.astype(np.float64) * q + 0.5 * eta * q * q).sum())
    charges = q.astype(np.float32)

    ce_at = np.asarray(charge_embed, np.float32)[species]          # [N,16]
    w_in = np.concatenate([charges[:, None], ce_at], axis=1)       # [N,17]
    w_tab = (w_in @ wwn).astype(np.float32)                        # [N,16]

    for c in range(N_CORES):
        xw_list[c][F_IN:H, :] = w_tab[send_pad[c]].T.astype(NPBF16)
        xw_list[c][F_IN:H, E_C:] = 0.0

    # ---- NEFF B ----
    ncB = _build_neff_b()
    in_b = [{"xw": xw_list[c], "vecw": vecw_list[c],
             "w1": w1b, "w2": w2b, "w3": w3b} for c in range(N_CORES)]
    resB = run_bass_kernel_spmd(ncB, in_b, core_ids=list(range(N_CORES)),
                                trace=trace)
    if trace:
        LAST_EXEC_NS["B"] = resB.exec_time_ns
        LAST_RES["B"] = resB

    x_out = np.empty((E, H), np.float32)
    for c in range(N_CORES):
        xo = resB.results[c]["xout"].reshape(E_P, H)
        x_out[c * E_C:(c + 1) * E_C] = xo[:E_C]

    return x_out, V, charges, pot


# revision 10
# speedup vs baseline: 4.3548x; 1.4865x over previous
"""AllegroQeq layer on 8 Trainium2 NeuronCores.

Structure:
  - host: shard 1.6M edges across 8 cores (200704 padded each), build a
    transposed feature stream xw [64, E_P] per core (rows 0:48 = x.T).
  - NEFF-A (per core): per-edge 2-layer MLP -> chis_e [E_P]  (DMA-bound).
  - host mid: bincount(senders) -> chis -> charges/pot/w table, fill
    xw rows 48:64 with w[senders].T.
  - NEFF-B (per core): 3-layer MLP on [x|w_s] + smoothing envelope,
    writes x_out rows in place (PE flip matmuls give row-major output).
  - V passes through untouched.
"""
import os
import sys
import types
import contextlib

sys.path.insert(0, "/opt/trn_rl_repo")

import numpy as np

import concourse.bass as bass
import concourse.bacc as bacc
import concourse.tile as tile
from concourse import mybir
from concourse.bass_utils import run_bass_kernel_spmd

import ml_dtypes

AF = mybir.ActivationFunctionType
ALU = mybir.AluOpType
FP32 = mybir.dt.float32
BF16 = mybir.dt.bfloat16
NPBF16 = ml_dtypes.bfloat16

N_CORES = 8
E = 1_600_000
N_ATOMS = 50_000
F_IN = 48
CE = 16
H = 64
S = 100
E_C = E // N_CORES            # 200000 edges per core
GROUPS = 98                   # groups of 2048 edges
E_P = GROUPS * 2048           # 200704 padded per-core edges

# Profiling side-channel for test.py (not used by the harness).
LAST_EXEC_NS = {}
LAST_RES = {}


def _install_axon_profile_shim():
    """Register the NTFF profile hook missing from the container's antenv
    stub, and neuter the S3 artifact upload. Best-effort."""
    try:
        if "antenv.axon_hooks" not in sys.modules:
            mod = types.ModuleType("antenv.axon_hooks")
            _hook = [None]
            mod.set_axon_ntff_profile_hook = lambda h: _hook.__setitem__(0, h)
            mod.get_axon_ntff_profile_hook = lambda: _hook[0]
            sys.modules["antenv.axon_hooks"] = mod
            import antenv
            antenv.axon_hooks = mod
        from antenv.axon_hooks import set_axon_ntff_profile_hook
        from trn_agent_boot.trn_boot import _ntff_profile_via_ctypes
        hook = _ntff_profile_via_ctypes("/opt/axon/libaxon_pjrt.so")
        if hook is None:
            return False
        set_axon_ntff_profile_hook(hook)
        import concourse.bass_utils as bu
        bu.upload_artifacts = lambda tmpdir: str(tmpdir)
        return True
    except Exception:
        return False


def _build_neff_a():
    """Per-edge chis MLP: chis_e = silu(x @ Wc1) @ Wc2  (Wc* pre-normalized)."""
    nc = bacc.Bacc("TRN2", target_bir_lowering=False, debug=False,
                   num_devices=N_CORES)
    xw = nc.dram_tensor("xw", [H, E_P], BF16, kind="ExternalInput")
    wc1 = nc.dram_tensor("wc1", [F_IN, 32], BF16, kind="ExternalInput")
    wc2 = nc.dram_tensor("wc2", [128, 4], BF16, kind="ExternalInput")
    ce_out = nc.dram_tensor("ce", [GROUPS, 4, 512], FP32, kind="ExternalOutput")

    with tile.TileContext(nc) as tc:
        with tc.tile_pool(name="sbuf", bufs=3) as sb, \
             tc.tile_pool(name="wpool", bufs=1) as wp, \
             tc.tile_pool(name="psum", bufs=2, space="PSUM") as pp:
            wc1_t = wp.tile([F_IN, 32], BF16)
            wc2_t = wp.tile([128, 4], BF16)
            zrow = wp.tile([1, 512], BF16)
            nc.sync.dma_start(wc1_t[:], wc1[:])
            nc.sync.dma_start(wc2_t[:], wc2[:])
            nc.vector.memset(zrow[:], 0.0)

            # pre-zero both psum4 slots so the [128,512] silu reads no junk
            pre = [pp.tile([128, 512], FP32, tag="psum4", name=f"psum4_pre{i}")
                   for i in range(2)]
            for t in pre:
                nc.tensor.matmul(t[:], lhsT=zrow[:, 0:128], rhs=zrow[:],
                                 start=True, stop=True)

            ce_v = ce_out[:].rearrange("g m j -> m g j")
            stgb = None
            for g in range(GROUPS):
                if g % 2 == 0:
                    xs = sb.tile([F_IN, 4096], BF16, tag="xs")
                    nc.sync.dma_start(xs[:],
                                      xw[0:F_IN, g * 2048:(g + 2) * 2048])
                xsl = xs[:, 2048 * (g % 2):2048 * (g % 2) + 2048]
                p4 = pp.tile([128, 512], FP32, tag="psum4")
                for m in range(4):
                    nc.tensor.matmul(p4[32 * m:32 * m + 32, :],
                                     lhsT=wc1_t[:],
                                     rhs=xsl[:, 512 * m:512 * m + 512],
                                     start=True, stop=True,
                                     tile_position=(0, 32 * m))
                h1s = sb.tile([128, 512], BF16, tag="h1s")
                nc.scalar.activation(h1s[:], p4[:], AF.Silu)
                pc = pp.tile([4, 512], FP32, tag="pchis")
                nc.tensor.matmul(pc[:], lhsT=wc2_t[:], rhs=h1s[:],
                                 start=True, stop=True)
                if g % 4 == 0:
                    stgb = sb.tile([4, 2048], FP32, tag="cstg")
                nc.vector.tensor_copy(stgb[:, 512 * (g % 4):512 * (g % 4) + 512],
                                      pc[:])
                if g % 4 == 3:
                    nc.sync.dma_start(ce_v[:, g - 3:g + 1, :], stgb[:])
    nc.compile()
    return nc


def _build_neff_b():
    """3-layer MLP on [x|w_s] with envelope; row-major output via flips."""
    nc = bacc.Bacc("TRN2", target_bir_lowering=False, debug=False,
                   num_devices=N_CORES)
    xw = nc.dram_tensor("xw", [H, E_P], BF16, kind="ExternalInput")
    vecw = nc.dram_tensor("vecw", [128, GROUPS * 48], FP32, kind="ExternalInput")
    w1 = nc.dram_tensor("w1", [H, H], BF16, kind="ExternalInput")
    w2 = nc.dram_tensor("w2", [128, H], BF16, kind="ExternalInput")
    w3 = nc.dram_tensor("w3", [128, H], BF16, kind="ExternalInput")
    xout = nc.dram_tensor("xout", [4 * GROUPS, 128, 256], FP32,
                          kind="ExternalOutput")

    NV = GROUPS * 16            # env cols (one per 4-edge slot)
    with tile.TileContext(nc) as tc:
        with tc.tile_pool(name="sbuf", bufs=3) as sb, \
             tc.tile_pool(name="wpool", bufs=1) as wp, \
             tc.tile_pool(name="psum", bufs=2, space="PSUM") as pp:
            w1_t = wp.tile([H, H], BF16)
            w2_t = wp.tile([128, H], BF16)
            w3_t = wp.tile([128, H], BF16)
            nc.sync.dma_start(w1_t[:], w1[:])
            nc.sync.dma_start(w2_t[:], w2[:])
            nc.sync.dma_start(w3_t[:], w3[:])

            # ---- prologue: envelope for ALL edges (one Sqrt table load) ----
            vt = wp.tile([128, GROUPS * 48], FP32)
            nc.sync.dma_start(vt[:], vecw[:])
            env_all = wp.tile([128, NV], FP32)
            u6 = wp.tile([128, NV], FP32)
            nslab = 2
            for s in range(nslab):
                W = NV // nslab
                vs = vt[:, s * 3 * W:(s + 1) * 3 * W]
                sq = sb.tile([128, 3 * W], FP32, tag="sq")
                nc.vector.tensor_tensor(sq[:], vs, vs, op=ALU.mult)
                u = env_all[:, s * W:(s + 1) * W]
                nc.vector.tensor_reduce(
                    u, sq[:].rearrange("p (e c) -> p e c", c=3),
                    axis=mybir.AxisListType.X, op=ALU.add)
                nc.scalar.activation(u, u, AF.Sqrt)
            # env = 1 + u^6 * (48u - 21u^2 - 28);  u clamped to 1
            nc.vector.tensor_scalar_min(env_all[:], env_all[:], 1.0)
            u2 = wp.tile([128, NV], FP32)
            nc.vector.tensor_tensor(u2[:], env_all[:], env_all[:], op=ALU.mult)
            nc.vector.tensor_tensor(u6[:], u2[:], u2[:], op=ALU.mult)
            nc.vector.tensor_tensor(u6[:], u6[:], u2[:], op=ALU.mult)
            nc.vector.tensor_scalar(u2[:], env_all[:], -21.0, 48.0,
                                    op0=ALU.mult, op1=ALU.add)
            nc.vector.tensor_tensor(u2[:], u2[:], env_all[:], op=ALU.mult)
            nc.vector.tensor_scalar_add(u2[:], u2[:], -28.0)
            nc.vector.tensor_tensor(env_all[:], u6[:], u2[:], op=ALU.mult)
            nc.vector.tensor_scalar(env_all[:], env_all[:], 1.0, None,
                                    op0=ALU.add)

            for g in range(GROUPS):
                xs = sb.tile([H, 2048], BF16, tag="xs")
                nc.sync.dma_start(xs[:], xw[:, g * 2048:(g + 1) * 2048])
                for cp in range(2):       # pairs of 512-edge chunks
                    p0 = pp.tile([128, 512], FP32, tag="p0")
                    nc.tensor.matmul(p0[0:64, :], lhsT=w1_t[:],
                                     rhs=xs[:, 1024 * cp:1024 * cp + 512],
                                     start=True, stop=True)
                    nc.tensor.matmul(p0[64:128, :], lhsT=w1_t[:],
                                     rhs=xs[:, 1024 * cp + 512:1024 * cp + 1024],
                                     start=True, stop=True,
                                     tile_position=(0, 64))
                    h1 = sb.tile([128, 512], BF16, tag="h1")
                    nc.scalar.activation(h1[:], p0[:], AF.Silu)
                    p1 = pp.tile([128, 512], FP32, tag="p1")
                    nc.tensor.matmul(p1[0:64, :], lhsT=w2_t[0:64, :],
                                     rhs=h1[0:64, :], start=True, stop=True)
                    nc.tensor.matmul(p1[64:128, :], lhsT=w2_t[64:128, :],
                                     rhs=h1[64:128, :],
                                     start=True, stop=True,
                                     tile_position=(64, 64))
                    h2 = sb.tile([128, 512], BF16, tag="h2")
                    nc.scalar.activation(h2[:], p1[:], AF.Silu)
                    for sub in range(2):
                        c = 2 * cp + sub
                        h2r = h2[64 * sub:64 * sub + 64, :].rearrange(
                            "f (m t) -> f t m", t=4)
                        p2 = pp.tile([128, 256], FP32, tag="p2")
                        for t in range(4):
                            nc.tensor.matmul(
                                p2[:, 64 * t:64 * t + 64],
                                lhsT=h2r[:, t, :],
                                rhs=w3_t[64 * sub:64 * sub + 64, :],
                                start=True, stop=True,
                                tile_position=(64 * sub, 0))
                        stg = sb.tile([128, 256], FP32, tag="stg")
                        ecol = 16 * g + 4 * c
                        nc.vector.tensor_tensor(
                            stg[:].rearrange("p (t f) -> p t f", t=4),
                            p2[:].rearrange("p (t f) -> p t f", t=4),
                            env_all[:, ecol:ecol + 4].to_broadcast([128, 4, 64]),
                            op=ALU.mult)
                        nc.sync.dma_start(xout[4 * g + c][:, :], stg[:])
    nc.compile()
    return nc


def _softplus(x):
    x = x.astype(np.float64)
    return np.log1p(np.exp(-np.abs(x))) + np.maximum(x, 0.0)


def kernel(vectors, x, V, senders, species, radius, hardness, charge_embed,
           chi_scale, g_scale, g_shift, W_chi1, W_chi2, W_w, W_x1, W_x2, W_x3):
    vectors = np.asarray(vectors, np.float32)
    x = np.asarray(x, np.float32)
    V = np.asarray(V)
    senders = np.asarray(senders)
    species = np.asarray(species)

    trace = bool(os.environ.get("BASS_KERNEL_TRACE"))
    if trace:
        trace = _install_axon_profile_shim()

    # ---- pre-normalized weights (match reference's W / sqrt(fan_in)) ----
    wc1n = (np.asarray(W_chi1, np.float32)
            / np.float32(np.sqrt(F_IN))).astype(np.float32)       # [48,16]
    wc2n = (np.asarray(W_chi2, np.float32) / np.float32(np.sqrt(CE))
            * np.float32(chi_scale)).astype(np.float32)           # [16,1]
    w1n = (np.asarray(W_x1, np.float32) / np.float32(np.sqrt(H))).astype(np.float32)
    w2n = (np.asarray(W_x2, np.float32) / np.float32(np.sqrt(H))).astype(np.float32)
    w3n = (np.asarray(W_x3, np.float32) / np.float32(np.sqrt(H))).astype(np.float32)
    wwn = (np.asarray(W_w, np.float32) / np.float32(np.sqrt(1 + CE))).astype(np.float32)

    wc1_pad = np.zeros((F_IN, 32), NPBF16)
    wc1_pad[:, :CE] = wc1n.astype(NPBF16)
    wc2_blk = np.zeros((128, 4), NPBF16)
    for m in range(4):
        wc2_blk[32 * m:32 * m + CE, m] = wc2n[:, 0].astype(NPBF16)
    w1b = w1n.astype(NPBF16)
    w2b = w2n.astype(NPBF16)
    w3b = w3n.astype(NPBF16)

    # ---- shard + host layouts ----
    xw_list, vecw_list, send_pad = [], [], []
    for c in range(N_CORES):
        sl = slice(c * E_C, (c + 1) * E_C)
        xw = np.zeros((H, E_P), NPBF16)
        xw[0:F_IN, :E_C] = x[sl].T.astype(NPBF16)
        xw_list.append(xw)
        vp = np.zeros((E_P, 3), np.float32)
        vp[:E_C] = vectors[sl]
        # vecw[p, g*48 + 4*cc + tt ... ] <- layout [128, G, 4cc, 4tt, 3]
        v5 = vp.reshape(GROUPS, 4, 128, 4, 3).transpose(2, 0, 1, 3, 4)
        vecw_list.append(np.ascontiguousarray(v5).reshape(128, GROUPS * 48))
        sp = np.zeros(E_P, np.int64)
        sp[:E_C] = senders[sl]
        send_pad.append(sp)

    # ---- NEFF A ----
    ncA = _build_neff_a()
    in_a = [{"xw": xw_list[c], "wc1": wc1_pad, "wc2": wc2_blk}
            for c in range(N_CORES)]
    resA = run_bass_kernel_spmd(ncA, in_a, core_ids=list(range(N_CORES)),
                                trace=trace)
    if trace:
        LAST_EXEC_NS["A"] = resA.exec_time_ns
        LAST_RES["A"] = resA

    # ---- host mid: segment sum + charge solve + w table ----
    chis = np.zeros(N_ATOMS, np.float64)
    for c in range(N_CORES):
        ce = resA.results[c]["ce"].reshape(E_P)
        chis += np.bincount(send_pad[c], weights=ce.astype(np.float64),
                            minlength=N_ATOMS)
    chis = chis.astype(np.float32)

    gammas = (np.asarray(radius, np.float32)[species] * np.float32(g_scale)
              + np.float32(g_shift))
    hard = _softplus(np.asarray(hardness, np.float32)[species]).astype(np.float32)
    eta = (hard.astype(np.float64) + 1.0 / gammas.astype(np.float64))
    q = (-chis.astype(np.float64) / eta)
    pot = np.float32((chis.astype(np.float64) * q + 0.5 * eta * q * q).sum())
    charges = q.astype(np.float32)

    ce_at = np.asarray(charge_embed, np.float32)[species]          # [N,16]
    w_in = np.concatenate([charges[:, None], ce_at], axis=1)       # [N,17]
    w_tab = (w_in @ wwn).astype(np.float32)                        # [N,16]

    for c in range(N_CORES):
        xw_list[c][F_IN:H, :] = w_tab[send_pad[c]].T.astype(NPBF16)
        xw_list[c][F_IN:H, E_C:] = 0.0

    # ---- NEFF B ----
    ncB = _build_neff_b()
    w2d = np.concatenate([w2b, w2b], axis=0)
    w3d = np.concatenate([w3b, w3b], axis=0)
    in_b = [{"xw": xw_list[c], "vecw": vecw_list[c],
             "w1": w1b, "w2": w2d, "w3": w3d} for c in range(N_CORES)]
    resB = run_bass_kernel_spmd(ncB, in_b, core_ids=list(range(N_CORES)),
                                trace=trace)
    if trace:
        LAST_EXEC_NS["B"] = resB.exec_time_ns
        LAST_RES["B"] = resB

    x_out = np.empty((E, H), np.float32)
    for c in range(N_CORES):
        xo = resB.results[c]["xout"].reshape(E_P, H)
        x_out[c * E_C:(c + 1) * E_C] = xo[:E_C]

    return x_out, V, charges, pot
